# revision 4
# baseline (speedup 1.0000x reference)
"""Trainium2 Bass kernel for LISTA (nn_LISTA_37976100831401).

Data-parallel sharding: batch 16384 -> 8 NeuronCores x 2048 rows.
We / S / theta are replicated on every core; no cross-device comm.

Per-core algorithm (all in the transposed [feature, batch] orientation):
  B.T  = We @ X.T                  (1024, 2048)
  Z0 = soft(B);  Z_{t+1} = soft(B + Z_t @ S.T), t = 0..15
  soft(x) = relu(x - theta) - relu(-x - theta) = x - clip(x, -theta, theta)

All matmul operands are bf16 (fp8 DoubleRow measured 233ns per 2-k-block
instruction vs bf16's 223ns per 1-k-block instruction - the PE is
column-rate-bound, so the accuracy-sufficient 3-product fp8 split is a
1.57x LOSS; plain fp8 sims at 0.09 rel err vs the 2e-2 gate).  The
kernel is PE-bound: 4160 matmuls x ~216ns issue stride ~= 899us.  The
rest is overhead trimming, informed by NTFF traces:

 * Head: ~7.2us fixed framework preamble, then DMA.  Critical bytes
   before the first b matmul: wet (0.5MB, sync queue) || chunk-0 of
   X.T (0.25MB, scalar+gpsimd queues); the rest of X.T (2KB+ row
   elements - 1KB-element DMAs are descriptor-rate-bound) follows on
   the same queues.  8 warmup matmuls on a memset scratch tile ramp
   the PE p-state (0.65->2.4GHz over ~3us) while DMAs fly.
 * b-phase: one PSUM read per tile (the B.T copy, needed anyway,
   ACT/DVE alternating), then soft-threshold from the bf16 SBUF copy
   split DVE/Pool.  GPSIMD(Pool) has no PSUM port so it only joins
   once data is in SBUF.  b(2)/b(3) groups are interleaved between
   step groups (a b burst is 8 x 447ns groups whose copies drain at
   ~350ns/tile-engine; bursting them stalled the PE ~0.6us per burst
   on PSUM bufs).  PSUM pool uses all 8 banks.
 * Tail: the final step threshold uses the clip path (DVE+Pool, no
   serial ACT relu pair) to shorten the last tile's drain chain, and
   Z goes out as bf16 (host upcasts; adds <=0.2% of scale) rotating
   over the sync/scalar/gpsimd DMA queues.

Layout: Z.T keeps the feature dim m=1024 on SBUF partitions (8 tiles
of 128) and batch on the free dim; C.T = S @ Z.T + B.T accumulates in
PSUM via psum[j,b] += ST[k][:, j128].T @ ZT[k][:, b] and the matmul
OUTPUT layout [j, b] equals the INPUT layout [k, b] of the next step,
so no transposes anywhere.  The host transposes the output (not on the
device critical path, same as the input-side X.T/We.T/S.T prep).

Engine balance per steady-state step tile (PE budget 8 matmuls ~1.74us):
  DVE:  ct = psum + B (fp32-in, ~690ns), z = af - df (bf16, ~424ns)
  ACT:  af = relu(ct - th), df = relu(-ct - th)  (2 x ~713ns)
"""

import time
from contextlib import ExitStack

import ml_dtypes
import numpy as np

import concourse.bacc as bacc
import concourse.mybir as mybir
import concourse.tile as tile
from concourse import bass_utils

FP32 = mybir.dt.float32
BF16 = mybir.dt.bfloat16
AL = mybir.AluOpType
AF = mybir.ActivationFunctionType

N_CORES = 8
B_TOTAL, N_IN, M = 16384, 256, 1024
B_CORE = B_TOTAL // N_CORES  # 2048
T_STEPS = 16                 # scan length in the reference
CHUNK = 512                  # batch columns per j-sweep (= PSUM bank cap)
N_CHUNKS = B_CORE // CHUNK   # 4
KT = M // 128                # 8 feature tiles of 128
NT = N_IN // 128             # 2 input-feature tiles
N_WARM = 8                   # PE p-state warmup matmuls


def _emit(ctx: ExitStack, tc: tile.TileContext, XT, WeT, ST, NTH, TH, Z):
    nc = tc.nc

    const_pool = ctx.enter_context(tc.tile_pool(name="const", bufs=1))
    psum_pool = ctx.enter_context(tc.tile_pool(name="psum", bufs=1, space="PSUM"))
    bt_pool = ctx.enter_context(tc.tile_pool(name="bt", bufs=1))
    zt_pool = ctx.enter_context(tc.tile_pool(name="zt", bufs=1))
    tmp_pool = ctx.enter_context(tc.tile_pool(name="tmp", bufs=1))
    out_pool = ctx.enter_context(tc.tile_pool(name="zout", bufs=1))

    # ---- constants -------------------------------------------------------
    nth = const_pool.tile([128, KT], FP32, name="nth")
    th = const_pool.tile([128, KT], FP32, name="th")
    wet = [const_pool.tile([128, M], BF16, name=f"wet{nt}") for nt in range(NT)]
    st = [const_pool.tile([128, M], BF16, name=f"st{kt}") for kt in range(KT)]
    # X.T as 2 full-row tiles [128, 2048]; chunks are free-dim slices.
    xt = [const_pool.tile([128, B_CORE], BF16, name=f"xt{nt}") for nt in range(NT)]
    warm = const_pool.tile([128, CHUNK], BF16, name="warm")

    bts = {}  # chunk -> [KT] tiles [128, CHUNK]   (B.T slab, bf16)
    zts = {}  # chunk -> [KT] tiles [128, CHUNK]   (current Z.T, bf16)

    def emit_head_dmas():
        # Critical path: wet (sync) + xt chunk 0 (scalar/gpsimd) gate the
        # first b matmul; st row-blocks are needed by step(0) (~25us in)
        # and ride the sync queue behind the weights.
        for nt in range(NT):
            nc.sync.dma_start(wet[nt][:], WeT[nt * 128 : (nt + 1) * 128, :])
        for nt, eng in ((0, nc.scalar), (1, nc.gpsimd)):
            r = slice(nt * 128, (nt + 1) * 128)
            eng.dma_start(xt[nt][:, 0:CHUNK], XT[r, 0:CHUNK])
            eng.dma_start(xt[nt][:, CHUNK:B_CORE], XT[r, CHUNK:B_CORE])
        nc.sync.dma_start(nth[:], NTH)
        nc.sync.dma_start(th[:], TH)
        for kt in range(KT):
            nc.sync.dma_start(st[kt][:], ST[kt * 128 : (kt + 1) * 128, :])

    def emit_warmup():
        # Ramp the PE while the head DMAs are in flight: matmuls on a
        # memset scratch tile into scratch psum groups, never consumed.
        nc.gpsimd.memset(warm[:], 0.0)
        for _ in range(N_WARM):
            ps = psum_pool.tile([128, CHUNK], FP32, name="psw", tag="mm", bufs=8)
            nc.tensor.matmul(ps[:], warm[:, 0:128], warm[:], start=True, stop=True)

    def b_tile(c, jt):
        # One B.T tile: K=256 matmul group, PSUM->SBUF copy, then
        # Z0 = soft(B) from the bf16 copy (clip+sub, DVE/Pool alternating).
        ps = psum_pool.tile([128, CHUNK], FP32, name="psb", tag="mm", bufs=8)
        for nt in range(NT):
            nc.tensor.matmul(
                ps[:],
                wet[nt][:, jt * 128 : (jt + 1) * 128],
                xt[nt][:, c * CHUNK : (c + 1) * CHUNK],
                start=(nt == 0),
                stop=(nt == NT - 1),
            )
        btile = bt_pool.tile([128, CHUNK], BF16, name="btile", tag=f"bt{jt}", bufs=4)
        if jt % 2 == 0:
            nc.scalar.copy(btile[:], ps[:])
        else:
            nc.vector.tensor_copy(btile[:], ps[:])
        eng = nc.vector if jt % 2 == 0 else nc.gpsimd
        cl = tmp_pool.tile([128, CHUNK], BF16, name="cl", tag="cl", bufs=4)
        eng.tensor_scalar(
            cl[:], btile[:], th[:, jt : jt + 1], nth[:, jt : jt + 1],
            op0=AL.min, op1=AL.max,
        )
        z0 = zt_pool.tile([128, CHUNK], BF16, name="z", tag=f"zt{jt}", bufs=6)
        eng.tensor_sub(z0[:], btile[:], cl[:])
        bts[c].append(btile)
        zts[c].append(z0)

    def b_phase(c):
        bts[c] = []
        zts[c] = []
        for jt in range(KT):
            b_tile(c, jt)

    def step_tile(c, jt, zcur, znew, final):
        ps = psum_pool.tile([128, CHUNK], FP32, name="pss", tag="mm", bufs=8)
        for kt in range(KT):
            nc.tensor.matmul(
                ps[:],
                st[kt][:, jt * 128 : (jt + 1) * 128],
                zcur[kt][:],
                start=(kt == 0),
                stop=(kt == KT - 1),
            )
        ct = tmp_pool.tile([128, CHUNK], BF16, name="ct", tag="ct", bufs=3)
        nc.vector.tensor_add(ct[:], ps[:], bts[c][jt][:])
        if final:
            # Clip path, DVE/Pool split: shorter drain chain than the
            # serial ACT relu pair, and ACT is idle at the tail anyway.
            e1, e2 = (nc.gpsimd, nc.vector) if jt % 2 == 0 else (nc.vector, nc.gpsimd)
            cl = tmp_pool.tile([128, CHUNK], BF16, name="cl", tag="cl", bufs=4)
            e1.tensor_scalar(
                cl[:], ct[:], th[:, jt : jt + 1], nth[:, jt : jt + 1],
                op0=AL.min, op1=AL.max,
            )
            zo = out_pool.tile([128, CHUNK], BF16, name="zo", tag="zo", bufs=3)
            e2.tensor_sub(zo[:], ct[:], cl[:])
            dma_eng = (nc.sync, nc.scalar, nc.gpsimd)[jt % 3]
            dma_eng.dma_start(
                Z[jt * 128 : (jt + 1) * 128, c * CHUNK : (c + 1) * CHUNK],
                zo[:],
            )
        else:
            af = tmp_pool.tile([128, CHUNK], BF16, name="af", tag="af", bufs=3)
            nc.scalar.activation(
                af[:], ct[:], AF.Relu, bias=nth[:, jt : jt + 1], scale=1.0
            )
            df = tmp_pool.tile([128, CHUNK], BF16, name="df", tag="df", bufs=3)
            nc.scalar.activation(
                df[:], ct[:], AF.Relu, bias=nth[:, jt : jt + 1], scale=-1.0
            )
            zn = zt_pool.tile([128, CHUNK], BF16, name="z", tag=f"zt{jt}", bufs=6)
            nc.vector.tensor_sub(zn[:], af[:], df[:])
            znew.append(zn)

    def step(c, final=False, b_chunk=None):
        # Z <- soft(B + Z @ S.T).  If b_chunk is given, interleave that
        # chunk's b groups between this step's groups (keeps the short
        # K=256 b groups from bursting ahead of their PSUM consumers).
        zcur = zts[c]
        znew = []
        if b_chunk is not None:
            bts[b_chunk] = []
            zts[b_chunk] = []
        for jt in range(KT):
            step_tile(c, jt, zcur, znew, final)
            if b_chunk is not None:
                b_tile(b_chunk, jt)
        if not final:
            zts[c] = znew

    emit_warmup()
    emit_head_dmas()
    b_phase(0)
    b_phase(1)
    step(0, b_chunk=2)
    step(1, b_chunk=3)
    step(2)
    step(3)
    for _ in range(T_STEPS - 2):
        for c in range(N_CHUNKS):
            step(c)
    for c in range(N_CHUNKS):
        step(c, final=True)


def build_nc():
    nc = bacc.Bacc("TRN2", target_bir_lowering=False, debug=False)
    XT = nc.dram_tensor("XT", [N_IN, B_CORE], BF16, kind="ExternalInput")
    WeT = nc.dram_tensor("WeT", [N_IN, M], BF16, kind="ExternalInput")
    ST = nc.dram_tensor("ST", [M, M], BF16, kind="ExternalInput")
    NTH = nc.dram_tensor("NTH", [128, KT], FP32, kind="ExternalInput")
    TH = nc.dram_tensor("TH", [128, KT], FP32, kind="ExternalInput")
    Z = nc.dram_tensor("Z", [M, B_CORE], BF16, kind="ExternalOutput")
    with tile.TileContext(nc) as tc:
        with ExitStack() as ctx:
            _emit(ctx, tc, XT.ap(), WeT.ap(), ST.ap(), NTH.ap(), TH.ap(), Z.ap())
    nc.compile()
    return nc


_NC_CACHE = None


def _get_nc():
    global _NC_CACHE
    if _NC_CACHE is None:
        _NC_CACHE = build_nc()
    return _NC_CACHE


def make_in_maps(X, We, S, theta):
    X = np.asarray(X, dtype=np.float32)
    WeT = np.ascontiguousarray(
        np.asarray(We, dtype=np.float32).T.astype(ml_dtypes.bfloat16)
    )
    ST = np.ascontiguousarray(
        np.asarray(S, dtype=np.float32).T.astype(ml_dtypes.bfloat16)
    )
    TH = np.ascontiguousarray(
        np.asarray(theta, dtype=np.float32).reshape(KT, 128).T
    )
    return [
        {
            "XT": np.ascontiguousarray(
                X[i * B_CORE : (i + 1) * B_CORE].T.astype(ml_dtypes.bfloat16)
            ),
            "WeT": WeT,
            "ST": ST,
            "NTH": np.ascontiguousarray(-TH),
            "TH": TH,
        }
        for i in range(N_CORES)
    ]


def gather_out(results):
    return np.concatenate(
        [
            np.ascontiguousarray(results[i]["Z"].T).astype(np.float32)
            for i in range(N_CORES)
        ],
        axis=0,
    )


def run(X, We, S, theta, trace=False, **trace_kwargs):
    nc = _get_nc()
    in_maps = make_in_maps(X, We, S, theta)
    # The PJRT compile callback can fail transiently ("CallFunctionObjArgs");
    # a retry in the same process succeeds.
    last_err = None
    for _attempt in range(3):
        try:
            res = bass_utils.run_bass_kernel_spmd(
                nc, in_maps, list(range(N_CORES)), trace=trace, **trace_kwargs
            )
            break
        except Exception as e:  # noqa: BLE001
            last_err = e
            time.sleep(2.0)
    else:
        raise last_err
    return gather_out(res.results), res


def kernel(X, We, S, theta):
    Z, _ = run(X, We, S, theta, trace=False)
    return Z


# revision 5
# speedup vs baseline: 1.0046x; 1.0046x over previous
"""Trainium2 Bass kernel for LISTA (nn_LISTA_37976100831401).

Data-parallel sharding: batch 16384 -> 8 NeuronCores x 2048 rows.
We / S / theta are replicated on every core; no cross-device comm.

Per-core algorithm (all in the transposed [feature, batch] orientation):
  B.T  = We @ X.T                  (1024, 2048)
  Z0 = soft(B);  Z_{t+1} = soft(B + Z_t @ S.T), t = 0..15
  soft(x) = relu(x - theta) - relu(-x - theta) = x - clip(x, -theta, theta)

All matmul operands are bf16 (fp8 DoubleRow measured 233ns per 2-k-block
instruction vs bf16's 223ns per 1-k-block instruction - the PE is
column-rate-bound, so the accuracy-sufficient 3-product fp8 split is a
1.57x LOSS; plain fp8 sims at 0.09 rel err vs the 2e-2 gate).  The
kernel is PE-bound: 4160 matmuls x ~216ns issue stride ~= 899us.  The
rest is overhead trimming, informed by NTFF traces:

 * Head: ~7.2us fixed framework preamble, then DMA.  Critical bytes
   before the first b matmul: wet (0.5MB, sync queue) || chunk-0 of
   X.T (0.25MB, scalar+gpsimd queues); the rest of X.T (2KB+ row
   elements - 1KB-element DMAs are descriptor-rate-bound) follows on
   the same queues.  8 warmup matmuls on a memset scratch tile ramp
   the PE p-state (0.65->2.4GHz over ~3us) while DMAs fly.
 * b-phase: one PSUM read per tile (the B.T copy, needed anyway,
   ACT/DVE alternating), then soft-threshold from the bf16 SBUF copy
   split DVE/Pool.  GPSIMD(Pool) has no PSUM port so it only joins
   once data is in SBUF.  b(2)/b(3) groups are interleaved between
   step groups (a b burst is 8 x 447ns groups whose copies drain at
   ~350ns/tile-engine; bursting them stalled the PE ~0.6us per burst
   on PSUM bufs).  PSUM pool uses all 8 banks.
 * Tail: the final step threshold uses the clip path (DVE+Pool, no
   serial ACT relu pair) to shorten the last tile's drain chain, and
   Z goes out as bf16 (host upcasts; adds <=0.2% of scale) rotating
   over the sync/scalar/gpsimd DMA queues.

Layout: Z.T keeps the feature dim m=1024 on SBUF partitions (8 tiles
of 128) and batch on the free dim; C.T = S @ Z.T + B.T accumulates in
PSUM via psum[j,b] += ST[k][:, j128].T @ ZT[k][:, b] and the matmul
OUTPUT layout [j, b] equals the INPUT layout [k, b] of the next step,
so no transposes anywhere.  The host transposes the output (not on the
device critical path, same as the input-side X.T/We.T/S.T prep).

Engine balance per steady-state step tile (PE budget 8 matmuls ~1.74us):
  DVE:  ct = psum + B (fp32-in, ~690ns), z = af - df (bf16, ~424ns)
  ACT:  af = relu(ct - th), df = relu(-ct - th)  (2 x ~713ns)
"""

import time
from contextlib import ExitStack

import ml_dtypes
import numpy as np

import concourse.bacc as bacc
import concourse.mybir as mybir
import concourse.tile as tile
from concourse import bass_utils

FP32 = mybir.dt.float32
BF16 = mybir.dt.bfloat16
AL = mybir.AluOpType
AF = mybir.ActivationFunctionType

N_CORES = 8
B_TOTAL, N_IN, M = 16384, 256, 1024
B_CORE = B_TOTAL // N_CORES  # 2048
T_STEPS = 16                 # scan length in the reference
CHUNK = 512                  # batch columns per j-sweep (= PSUM bank cap)
N_CHUNKS = B_CORE // CHUNK   # 4
KT = M // 128                # 8 feature tiles of 128
NT = N_IN // 128             # 2 input-feature tiles
N_WARM = 8                   # PE p-state warmup matmuls


def _emit(ctx: ExitStack, tc: tile.TileContext, XT, WeT, ST, NTH, TH, Z):
    nc = tc.nc

    const_pool = ctx.enter_context(tc.tile_pool(name="const", bufs=1))
    psum_pool = ctx.enter_context(tc.tile_pool(name="psum", bufs=1, space="PSUM"))
    bt_pool = ctx.enter_context(tc.tile_pool(name="bt", bufs=1))
    zt_pool = ctx.enter_context(tc.tile_pool(name="zt", bufs=1))
    tmp_pool = ctx.enter_context(tc.tile_pool(name="tmp", bufs=1))
    out_pool = ctx.enter_context(tc.tile_pool(name="zout", bufs=1))

    # ---- constants -------------------------------------------------------
    nth = const_pool.tile([128, KT], FP32, name="nth")
    th = const_pool.tile([128, KT], FP32, name="th")
    wet = [const_pool.tile([128, M], BF16, name=f"wet{nt}") for nt in range(NT)]
    st = [const_pool.tile([128, M], BF16, name=f"st{kt}") for kt in range(KT)]
    # X.T as 2 full-row tiles [128, 2048]; chunks are free-dim slices.
    xt = [const_pool.tile([128, B_CORE], BF16, name=f"xt{nt}") for nt in range(NT)]
    warm = const_pool.tile([128, CHUNK], BF16, name="warm")

    bts = {}  # chunk -> [KT] tiles [128, CHUNK]   (B.T slab, bf16)
    zts = {}  # chunk -> [KT] tiles [128, CHUNK]   (current Z.T, bf16)

    def emit_head_dmas():
        # Critical path: wet (sync) + xt chunk 0 (scalar/gpsimd) gate the
        # first b matmul; st row-blocks are needed by step(0) (~25us in)
        # and ride the sync queue behind the weights.
        for nt in range(NT):
            nc.sync.dma_start(wet[nt][:], WeT[nt * 128 : (nt + 1) * 128, :])
        for nt, eng in ((0, nc.scalar), (1, nc.gpsimd)):
            r = slice(nt * 128, (nt + 1) * 128)
            eng.dma_start(xt[nt][:, 0:CHUNK], XT[r, 0:CHUNK])
            eng.dma_start(xt[nt][:, CHUNK:B_CORE], XT[r, CHUNK:B_CORE])
        nc.sync.dma_start(nth[:], NTH)
        nc.sync.dma_start(th[:], TH)
        for kt in range(KT):
            nc.sync.dma_start(st[kt][:], ST[kt * 128 : (kt + 1) * 128, :])

    def emit_warmup():
        # Ramp the PE while the head DMAs are in flight: matmuls on a
        # memset scratch tile into scratch psum groups, never consumed.
        nc.gpsimd.memset(warm[:], 0.0)
        for _ in range(N_WARM):
            ps = psum_pool.tile([128, CHUNK], FP32, name="psw", tag="mm", bufs=8)
            nc.tensor.matmul(ps[:], warm[:, 0:128], warm[:], start=True, stop=True)

    def b_tile(c, jt):
        # One B.T tile: K=256 matmul group, PSUM->SBUF copy, then
        # Z0 = soft(B) from the bf16 copy (clip+sub, DVE/Pool alternating).
        ps = psum_pool.tile([128, CHUNK], FP32, name="psb", tag="mm", bufs=8)
        for nt in range(NT):
            nc.tensor.matmul(
                ps[:],
                wet[nt][:, jt * 128 : (jt + 1) * 128],
                xt[nt][:, c * CHUNK : (c + 1) * CHUNK],
                start=(nt == 0),
                stop=(nt == NT - 1),
            )
        btile = bt_pool.tile([128, CHUNK], BF16, name="btile", tag=f"bt{jt}", bufs=4)
        if jt % 2 == 0:
            nc.scalar.copy(btile[:], ps[:])
        else:
            nc.vector.tensor_copy(btile[:], ps[:])
        eng = nc.vector if jt % 2 == 0 else nc.gpsimd
        cl = tmp_pool.tile([128, CHUNK], BF16, name="cl", tag="cl", bufs=4)
        eng.tensor_scalar(
            cl[:], btile[:], th[:, jt : jt + 1], nth[:, jt : jt + 1],
            op0=AL.min, op1=AL.max,
        )
        z0 = zt_pool.tile([128, CHUNK], BF16, name="z", tag=f"zt{jt}", bufs=6)
        eng.tensor_sub(z0[:], btile[:], cl[:])
        bts[c].append(btile)
        zts[c].append(z0)

    def b_phase(c):
        bts[c] = []
        zts[c] = []
        for jt in range(KT):
            b_tile(c, jt)

    def step_tile(c, jt, zcur, znew, final):
        ps = psum_pool.tile([128, CHUNK], FP32, name="pss", tag="mm", bufs=8)
        for kt in range(KT):
            nc.tensor.matmul(
                ps[:],
                st[kt][:, jt * 128 : (jt + 1) * 128],
                zcur[kt][:],
                start=(kt == 0),
                stop=(kt == KT - 1),
            )
        ct = tmp_pool.tile([128, CHUNK], BF16, name="ct", tag="ct", bufs=3)
        nc.vector.tensor_add(ct[:], ps[:], bts[c][jt][:])
        if final:
            # Mixed paths, no Pool (Pool ops are ~2.7x slower than DVE and
            # stretched the drain chain): ACT relu pair on even tiles, DVE
            # clip on odd tiles -- so the last tile (jt=7) drains through
            # DVE + the idle sync queue with the shortest chain.
            zo = out_pool.tile([128, CHUNK], BF16, name="zo", tag="zo", bufs=3)
            if jt % 2 == 0:
                af = tmp_pool.tile([128, CHUNK], BF16, name="af", tag="af", bufs=3)
                nc.scalar.activation(
                    af[:], ct[:], AF.Relu, bias=nth[:, jt : jt + 1], scale=1.0
                )
                df = tmp_pool.tile([128, CHUNK], BF16, name="df", tag="df", bufs=3)
                nc.scalar.activation(
                    df[:], ct[:], AF.Relu, bias=nth[:, jt : jt + 1], scale=-1.0
                )
                nc.vector.tensor_sub(zo[:], af[:], df[:])
            else:
                cl = tmp_pool.tile([128, CHUNK], BF16, name="cl", tag="cl", bufs=4)
                nc.vector.tensor_scalar(
                    cl[:], ct[:], th[:, jt : jt + 1], nth[:, jt : jt + 1],
                    op0=AL.min, op1=AL.max,
                )
                nc.vector.tensor_sub(zo[:], ct[:], cl[:])
            dma_eng = nc.scalar if jt % 2 == 0 else nc.sync
            dma_eng.dma_start(
                Z[jt * 128 : (jt + 1) * 128, c * CHUNK : (c + 1) * CHUNK],
                zo[:],
            )
        else:
            af = tmp_pool.tile([128, CHUNK], BF16, name="af", tag="af", bufs=3)
            nc.scalar.activation(
                af[:], ct[:], AF.Relu, bias=nth[:, jt : jt + 1], scale=1.0
            )
            df = tmp_pool.tile([128, CHUNK], BF16, name="df", tag="df", bufs=3)
            nc.scalar.activation(
                df[:], ct[:], AF.Relu, bias=nth[:, jt : jt + 1], scale=-1.0
            )
            zn = zt_pool.tile([128, CHUNK], BF16, name="z", tag=f"zt{jt}", bufs=6)
            nc.vector.tensor_sub(zn[:], af[:], df[:])
            znew.append(zn)

    def step(c, final=False, b_chunk=None):
        # Z <- soft(B + Z @ S.T).  If b_chunk is given, interleave that
        # chunk's b groups between this step's groups (keeps the short
        # K=256 b groups from bursting ahead of their PSUM consumers).
        zcur = zts[c]
        znew = []
        if b_chunk is not None:
            bts[b_chunk] = []
            zts[b_chunk] = []
        for jt in range(KT):
            step_tile(c, jt, zcur, znew, final)
            if b_chunk is not None:
                b_tile(b_chunk, jt)
        if not final:
            zts[c] = znew

    emit_warmup()
    emit_head_dmas()
    b_phase(0)
    b_phase(1)
    step(0, b_chunk=2)
    step(1, b_chunk=3)
    step(2)
    step(3)
    for _ in range(T_STEPS - 2):
        for c in range(N_CHUNKS):
            step(c)
    for c in range(N_CHUNKS):
        step(c, final=True)


def build_nc():
    nc = bacc.Bacc("TRN2", target_bir_lowering=False, debug=False)
    XT = nc.dram_tensor("XT", [N_IN, B_CORE], BF16, kind="ExternalInput")
    WeT = nc.dram_tensor("WeT", [N_IN, M], BF16, kind="ExternalInput")
    ST = nc.dram_tensor("ST", [M, M], BF16, kind="ExternalInput")
    NTH = nc.dram_tensor("NTH", [128, KT], FP32, kind="ExternalInput")
    TH = nc.dram_tensor("TH", [128, KT], FP32, kind="ExternalInput")
    Z = nc.dram_tensor("Z", [M, B_CORE], BF16, kind="ExternalOutput")
    with tile.TileContext(nc) as tc:
        with ExitStack() as ctx:
            _emit(ctx, tc, XT.ap(), WeT.ap(), ST.ap(), NTH.ap(), TH.ap(), Z.ap())
    nc.compile()
    return nc


_NC_CACHE = None


def _get_nc():
    global _NC_CACHE
    if _NC_CACHE is None:
        _NC_CACHE = build_nc()
    return _NC_CACHE


def make_in_maps(X, We, S, theta):
    X = np.asarray(X, dtype=np.float32)
    WeT = np.ascontiguousarray(
        np.asarray(We, dtype=np.float32).T.astype(ml_dtypes.bfloat16)
    )
    ST = np.ascontiguousarray(
        np.asarray(S, dtype=np.float32).T.astype(ml_dtypes.bfloat16)
    )
    TH = np.ascontiguousarray(
        np.asarray(theta, dtype=np.float32).reshape(KT, 128).T
    )
    return [
        {
            "XT": np.ascontiguousarray(
                X[i * B_CORE : (i + 1) * B_CORE].T.astype(ml_dtypes.bfloat16)
            ),
            "WeT": WeT,
            "ST": ST,
            "NTH": np.ascontiguousarray(-TH),
            "TH": TH,
        }
        for i in range(N_CORES)
    ]


def gather_out(results):
    return np.concatenate(
        [
            np.ascontiguousarray(results[i]["Z"].T).astype(np.float32)
            for i in range(N_CORES)
        ],
        axis=0,
    )


def run(X, We, S, theta, trace=False, **trace_kwargs):
    nc = _get_nc()
    in_maps = make_in_maps(X, We, S, theta)
    # The PJRT compile callback can fail transiently ("CallFunctionObjArgs");
    # a retry in the same process succeeds.
    last_err = None
    for _attempt in range(3):
        try:
            res = bass_utils.run_bass_kernel_spmd(
                nc, in_maps, list(range(N_CORES)), trace=trace, **trace_kwargs
            )
            break
        except Exception as e:  # noqa: BLE001
            last_err = e
            time.sleep(2.0)
    else:
        raise last_err
    return gather_out(res.results), res


def kernel(X, We, S, theta):
    Z, _ = run(X, We, S, theta, trace=False)
    return Z
